# revision 23
# baseline (speedup 1.0000x reference)
"""BiGAT (2-omic projection + GATv2 conv + ELU) as a distributed Bass/Tile
kernel for 8 Trainium2 NeuronCores.

Strategy (graph/data parallel, per the sharding hint):
  - Nodes are permuted so core c owns a contiguous block of NPC rows:
    [mrna rows c*3125 .. +3125][pad][mirna rows 25000+c*3125 .. +3125][pad].
  - Phase A (per core): project the core's node shard (x @ Wp + bp), then
    xl = h @ Wl, xr = h @ Wr; write row-major fp16 xl/xr tables to DRAM.
    Inputs arrive HOST-TRANSPOSED ([D, rows] fp16) so no on-chip input
    transposes are needed; only the small xl/xr outputs are transposed
    back to row-major on the PE.
  - Phase B: AllGather the xl shards -> full xl table on every core.
  - Phase C (per core): edges grouped per 128-node dst block, gathered in
    merged multi-block dma_gathers. Per block: leaky-relu logits via
    fp16 2x-rate DVE ops, e4 = att . g via mult + binary reduction tree,
    exp on the Activation engine (expanded over C so the msg multiply
    stays packed-2x), one-hot built packed (is_equal against a repeated
    iota, broadcast over the *dst* axis so the last dim stays packed),
    and a one-hot matmul scatter-add into PSUM [dst, H*C | denom].
    Softmax normalization happens post-aggregation (identical math).
  - Epilogue: batched normalize + bias + ELU over all blocks, fp16 out.

Edge bookkeeping (host-side, integer-only): same as before — per
(core, dst-block) buckets, lo/hi sections for int16 gather indices,
padded to 128-edge tiles with dummy slots (row 0, dstmod sentinel 999).
"""

import sys
import numpy as np

sys.path.insert(0, "/opt/trn_rl_repo")

P = 128
H, C = 4, 32
HC = H * C
NEG_SLOPE = 0.2
E_CLAMP = 13.0     # safety clamp, above data max logit (~11.7)
E_SHIFT = -5.0     # exp(e + E_SHIFT): keeps ex, ex*xl, denom in fp16 range
MERGE = 4          # dst blocks per merged dma_gather


def configure(cores=8, n1=25000, n2=25000, d1=2000, d2=500,
              n1pad=3200, n2pad=3200, split=32768, ng=512):
    global CORES, N1, N2, D1, D2, N1PC, N2PC, N1PAD, N2PAD
    global NPC, NB, NTOT, SPLIT, NG, D1PAD, D2PAD
    CORES, N1, N2, D1, D2 = cores, n1, n2, d1, d2
    N1PC, N2PC = N1 // CORES, N2 // CORES
    N1PAD, N2PAD = n1pad, n2pad
    assert N1PC <= N1PAD and N2PC <= N2PAD
    assert N1PAD % 128 == 0 and N2PAD % 128 == 0
    NPC = N1PAD + N2PAD
    NB = NPC // 128
    NTOT = CORES * NPC
    SPLIT = split
    NG = ng
    D1PAD = (D1 + 127) // 128 * 128
    D2PAD = (D2 + 127) // 128 * 128


configure()


# ---------------------------------------------------------------------------
# host-side integer prep
# ---------------------------------------------------------------------------

def _new_ids(n):
    """Map original node ids to permuted-padded ids."""
    n = np.asarray(n)
    is1 = n < N1
    c = np.where(is1, n // N1PC, (n - N1) // N2PC)
    slot = np.where(is1, n % N1PC, N1PAD + (n - N1) % N2PC)
    return c * NPC + slot


def _wrap_idx(arr):
    """int16 index list [L] -> dma_gather layout [128, L//16]."""
    L = arr.shape[0]
    assert L % 16 == 0
    w = arr.reshape(L // 16, 16).T.astype(np.int16)  # [16, L/16]
    return np.tile(w, (8, 1))                        # [128, L/16]


def prep_edges(edge_index):
    """Returns per-core gather-index / dstmod arrays + (F_LO, F_HI)."""
    src, dst = edge_index[0].astype(np.int64), edge_index[1].astype(np.int64)
    s_new = _new_ids(src)
    d_new = _new_ids(dst)
    core = d_new // NPC
    dl = d_new - core * NPC
    blk = dl // 128
    lo = (s_new < SPLIT).astype(np.int64)

    # order edges by (core, blk, hi-section, arbitrary)
    order = np.lexsort((s_new, 1 - lo, blk, core))
    s_new, d_new, core, dl, blk, lo = (
        a[order] for a in (s_new, d_new, core, dl, blk, lo))

    # per (core, blk) counts of lo/hi
    key = core * NB + blk
    n_lo = np.zeros(CORES * NB, np.int64)
    n_hi = np.zeros(CORES * NB, np.int64)
    np.add.at(n_lo, key, lo)
    np.add.at(n_hi, key, 1 - lo)

    # per-block tile counts: max over cores (SPMD shares one program)
    nlo2 = n_lo.reshape(CORES, NB)
    nhi2 = n_hi.reshape(CORES, NB)
    F_lo_b = [int(v) for v in ((nlo2 + 127) // 128).max(axis=0)]
    F_hi_b = [int(v) for v in ((nhi2 + 127) // 128).max(axis=0)]
    F_b = [a + b for a, b in zip(F_lo_b, F_hi_b)]
    SLO, SHI, SF = sum(F_lo_b), sum(F_hi_b), sum(F_b)
    OLO = np.concatenate([[0], np.cumsum(F_lo_b)])
    OHI = np.concatenate([[0], np.cumsum(F_hi_b)])
    OF = np.concatenate([[0], np.cumsum(F_b)])

    gi_lo = np.zeros((CORES, SLO * 128), np.int16)
    gi_hi = np.zeros((CORES, SHI * 128), np.int16)
    gi_xr = np.zeros((CORES, SF * 128), np.int16)
    dmod = np.full((CORES, SF, 128), 999.0, np.float32)

    bounds = np.searchsorted(key, np.arange(CORES * NB + 1))
    for k in range(CORES * NB):
        c, b = divmod(k, NB)
        a0, a1 = bounds[k], bounds[k + 1]
        nl = int(n_lo[k]); nh = int(n_hi[k])
        assert a1 - a0 == nl + nh
        sl = s_new[a0:a0 + nl]
        sh = s_new[a0 + nl:a1] - SPLIT
        dloc = dl[a0:a1]
        mod = (dloc % 128).astype(np.float32)
        flo0 = int(OLO[b]) * 128
        fhi0 = int(OHI[b]) * 128
        ff0 = int(OF[b]) * 128
        gi_lo[c, flo0:flo0 + nl] = sl
        gi_hi[c, fhi0:fhi0 + nh] = sh
        gi_xr[c, ff0:ff0 + nl] = dloc[:nl]
        gi_xr[c, ff0 + F_lo_b[b] * 128:ff0 + F_lo_b[b] * 128 + nh] = dloc[nl:]
        dm = dmod[c].reshape(SF * 128)
        dm[ff0:ff0 + nl] = mod[:nl]
        dm[ff0 + F_lo_b[b] * 128:ff0 + F_lo_b[b] * 128 + nh] = mod[nl:]

    import ml_dtypes
    out = []
    dgrid = np.arange(128, dtype=np.float32)
    for c in range(CORES):
        glo = _wrap_idx(gi_lo[c])
        ghi = _wrap_idx(gi_hi[c])
        gxr = _wrap_idx(gi_xr[c])
        # fp8 one-hot, matmul-lhsT layout: ohp[p, tg*128 + d] =
        # (dmod[c, tg, p] == d); sentinel 999 rows give all-zero columns.
        oh = (dmod[c].transpose(1, 0)[:, :, None] == dgrid).astype(
            ml_dtypes.float8_e4m3fn).reshape(128, SF * 128)
        out.append(dict(gilo=glo, gihi=ghi, gixr=gxr,
                        ohp=np.ascontiguousarray(oh)))
    return out, F_lo_b, F_hi_b


def prep_shards(x_mrna, x_mirna):
    """Per-core HOST-TRANSPOSED padded fp16 feature shards [Dpad, Npad]."""
    shards = []
    for c in range(CORES):
        xm = np.zeros((D1PAD, N1PAD), np.float16)
        xm[:D1, :N1PC] = x_mrna[c * N1PC:(c + 1) * N1PC].T
        xr_ = np.zeros((D2PAD, N2PAD), np.float16)
        xr_[:D2, :N2PC] = x_mirna[c * N2PC:(c + 1) * N2PC].T
        shards.append((xm, xr_))
    return shards


# ---------------------------------------------------------------------------
# program builder
# ---------------------------------------------------------------------------

def build_program(F_lo_b, F_hi_b, phases="abce"):
    import concourse.bass as bass
    import concourse.mybir as mybir
    import concourse.tile as tile
    from concourse import bacc
    from concourse.masks import make_identity

    dt = mybir.dt
    f32 = dt.float32
    f16 = dt.float16
    Alu = mybir.AluOpType
    Act = mybir.ActivationFunctionType
    F_b = [a + b for a, b in zip(F_lo_b, F_hi_b)]
    SLO, SHI, SF = sum(F_lo_b), sum(F_hi_b), sum(F_b)
    OLO = [0]
    OHI = [0]
    OF = [0]
    for b in range(NB):
        OLO.append(OLO[-1] + F_lo_b[b])
        OHI.append(OHI[-1] + F_hi_b[b])
        OF.append(OF[-1] + F_b[b])
    FMAX = max(F_b)
    FLOMAX = max(F_lo_b)
    FHIMAX = max(F_hi_b)
    K1 = D1PAD // 128
    K2 = D2PAD // 128

    nc = bacc.Bacc("TRN2", target_bir_lowering=False, debug=False,
                   num_devices=CORES)

    # --- I/O ---------------------------------------------------------------
    xm = nc.dram_tensor("xm", [D1PAD, N1PAD], f16, kind="ExternalInput")
    xmi = nc.dram_tensor("xmi", [D2PAD, N2PAD], f16, kind="ExternalInput")
    wp1 = nc.dram_tensor("wp1", [D1PAD, P], f16, kind="ExternalInput")
    bp1 = nc.dram_tensor("bp1", [P, 1], f32, kind="ExternalInput")
    wp2 = nc.dram_tensor("wp2", [D2PAD, P], f16, kind="ExternalInput")
    bp2 = nc.dram_tensor("bp2", [P, 1], f32, kind="ExternalInput")
    wl = nc.dram_tensor("wl", [P, HC], f16, kind="ExternalInput")
    wr = nc.dram_tensor("wr", [P, HC], f16, kind="ExternalInput")
    attb = nc.dram_tensor("attb", [128, 128], f16, kind="ExternalInput")
    bgat = nc.dram_tensor("bgat", [128, 128], f16, kind="ExternalInput")
    gilo = nc.dram_tensor("gilo", [128, SLO * 8], dt.int16,
                          kind="ExternalInput")
    gihi = nc.dram_tensor("gihi", [128, SHI * 8], dt.int16,
                          kind="ExternalInput")
    gixr = nc.dram_tensor("gixr", [128, SF * 8], dt.int16,
                          kind="ExternalInput")
    ohp = nc.dram_tensor("ohp", [128, SF * 128], dt.float8e4,
                         kind="ExternalInput")
    outp = nc.dram_tensor("outp", [NPC, HC], f16, kind="ExternalOutput")

    xl_loc = nc.dram_tensor("xl_loc", [NPC, HC], f16)
    xr_loc = nc.dram_tensor("xr_loc", [NPC, HC], f16)
    xl_full = nc.dram_tensor("xl_full", [NTOT, HC], f16,
                             addr_space="Shared" if CORES > 4 else "Local")

    with tile.TileContext(nc, num_cores=CORES) as tc:
        with tc.tile_pool(name="const", bufs=1) as cst:

            ident = cst.tile([128, 128], f16)
            make_identity(nc, ident[:])

            # weights resident in SBUF
            wp1_sb = cst.tile([128, K1 * 128], f16)
            nc.sync.dma_start(
                wp1_sb[:].rearrange("d (k p) -> d k p", k=K1),
                wp1.ap().rearrange("(k d) p -> d k p", d=128))
            wp2_sb = cst.tile([128, K2 * 128], f16)
            nc.sync.dma_start(
                wp2_sb[:].rearrange("d (k p) -> d k p", k=K2),
                wp2.ap().rearrange("(k d) p -> d k p", d=128))
            bp1_sb = cst.tile([128, 1], f32)
            nc.sync.dma_start(bp1_sb[:], bp1.ap())
            bp2_sb = cst.tile([128, 1], f32)
            nc.sync.dma_start(bp2_sb[:], bp2.ap())
            wl_sb = cst.tile([128, HC], f16)
            nc.sync.dma_start(wl_sb[:], wl.ap())
            wr_sb = cst.tile([128, HC], f16)
            nc.sync.dma_start(wr_sb[:], wr.ap())

            # ---------------- phase A: projections -------------------------
            pa_ctx = tc.tile_pool(name="pa", bufs=2)
            pa = pa_ctx.__enter__()
            pa_ps_ctx = tc.tile_pool(name="pa_ps", bufs=2, space="PSUM")
            pa_ps = pa_ps_ctx.__enter__()
            pa_tps_ctx = tc.tile_pool(name="pa_tps", bufs=4, space="PSUM")
            pa_tps = pa_tps_ctx.__enter__()
            hp_ctx = tc.tile_pool(name="hp", bufs=1)
            hp = hp_ctx.__enter__()
            hT_all = hp.tile([128, NPC], f16)

            def _emit_proj(w_sb, table, g0, ng, nt, row0):
                xps = pa_ps.tile([128, NG], f32, tag="xps")
                nc.tensor.matmul(xps[:, :ng], lhsT=w_sb[:],
                                 rhs=hT_all[:, row0 + g0:row0 + g0 + ng],
                                 start=True, stop=True)
                xsb = pa.tile([128, NG], f16, tag="xsb")
                nc.scalar.activation(xsb[:, :ng], xps[:, :ng], Act.Copy)
                rsb = pa.tile([128, NG], f16, tag="rsb")
                for ti in range(nt):
                    tp = pa_tps.tile([128, 128], f16, tag="tp")
                    nc.tensor.transpose(
                        tp[:], xsb[:, ti * 128:(ti + 1) * 128], ident[:])
                    nc.scalar.activation(
                        rsb[:, ti * 128:(ti + 1) * 128], tp[:], Act.Copy)
                r0 = row0 + g0
                nc.sync.dma_start(
                    table.ap()[r0:r0 + ng, :]
                    .rearrange("(t p) j -> p t j", p=128),
                    rsb[:, :ng].rearrange("p (t j) -> p t j", j=HC))

            # pass 1: h projection + xl table (gates the AllGather)
            for sec, (xdram, K, wp_sb, bp_sb, row0, npad) in enumerate([
                    (xm, K1, wp1_sb, bp1_sb, 0, N1PAD),
                    (xmi, K2, wp2_sb, bp2_sb, N1PAD, N2PAD)] if "a" in phases
                    else []):
                for g0 in range(0, npad, NG):
                    ng = min(NG, npad - g0)
                    nt = ng // 128
                    # load transposed input chunks [d, k, ng] in one DMA
                    xT = pa.tile([128, K * NG], f16, tag="xT")
                    nc.sync.dma_start(
                        xT[:, :K * ng].rearrange("d (k n) -> d k n", k=K),
                        xdram.ap().rearrange("(k d) n -> d k n", d=128)
                        [:, :, g0:g0 + ng])
                    hps = pa_ps.tile([128, NG], f32, tag="hps")
                    for i in range(K):
                        nc.tensor.matmul(
                            hps[:, :ng],
                            lhsT=wp_sb[:, i * 128:(i + 1) * 128],
                            rhs=xT[:, i * ng:(i + 1) * ng],
                            start=(i == 0), stop=(i == K - 1))
                    # bias add + fp16 convert on the Activation engine
                    nc.scalar.activation(hT_all[:, row0 + g0:row0 + g0 + ng],
                                         hps[:, :ng],
                                         Act.Identity, bias=bp_sb[:, 0:1])
                    _emit_proj(wl_sb, xl_loc, g0, ng, nt, row0)

            # ---------------- phase B: halo exchange -----------------------
            if "b" in phases:
                nc.gpsimd.collective_compute(
                    "AllGather", Alu.bypass,
                    ins=[xl_loc.ap()],
                    outs=[xl_full.ap()],
                    replica_groups=[list(range(CORES))])

            # pass 2: xr table — overlaps the AllGather (PE/Act/SP engines)
            if "a" in phases:
                for row0, npad in ((0, N1PAD), (N1PAD, N2PAD)):
                    for g0 in range(0, npad, NG):
                        ng = min(NG, npad - g0)
                        _emit_proj(wr_sb, xr_loc, g0, ng, ng // 128, row0)
            hp_ctx.__exit__(None, None, None)
            pa_tps_ctx.__exit__(None, None, None)
            pa_ps_ctx.__exit__(None, None, None)
            pa_ctx.__exit__(None, None, None)

            # ---------------- phase C: edge processing ---------------------
            gilo_sb = cst.tile([128, SLO * 8], dt.int16)
            nc.sync.dma_start(gilo_sb[:], gilo.ap())
            gihi_sb = cst.tile([128, SHI * 8], dt.int16)
            nc.sync.dma_start(gihi_sb[:], gihi.ap())
            gixr_sb = cst.tile([128, SF * 8], dt.int16)
            nc.sync.dma_start(gixr_sb[:], gixr.ap())
            attb_sb = cst.tile([128, 128], f16)
            nc.sync.dma_start(attb_sb[:], attb.ap())
            bgat_sb = cst.tile([128, 128], f16)
            nc.sync.dma_start(bgat_sb[:], bgat.ap())
            eshift_sb = cst.tile([128, 1], f32)
            nc.vector.memset(eshift_sb[:], E_SHIFT)

            NMG = (NB + MERGE - 1) // MERGE  # merged gather groups

            stgp_ctx = tc.tile_pool(name="stgp", bufs=1)
            stgp = stgp_ctx.__enter__()
            stg = stgp.tile([128, NB * 132], f16, tag="stg")
            gp_ctx = tc.tile_pool(name="gp", bufs=2)
            gp = gp_ctx.__enter__()
            pc_ctx = tc.tile_pool(name="pc", bufs=3)
            pc = pc_ctx.__enter__()
            pc_ps_ctx = tc.tile_pool(name="pc_ps", bufs=4, space="PSUM")
            pc_ps = pc_ps_ctx.__enter__()
            if True:
                pending = []

                def _stage1(b, b0, xlo, xhi, xrb):
                    F_LO, F_HI, F = F_lo_b[b], F_hi_b[b], F_b[b]
                    o_lo = (OLO[b] - OLO[b0]) * 128
                    o_hi = (OHI[b] - OHI[b0]) * 128
                    o_f = (OF[b] - OF[b0]) * 128
                    lo_ap = xlo[:, o_lo:o_lo + F_LO * 128]
                    hi_ap = xhi[:, o_hi:o_hi + F_HI * 128]
                    xr_ap = xrb[:, o_f:o_f + F * 128]
                    # u = xl[src] + xr[dst]   (2x fp16)
                    ut = pc.tile([128, FMAX * 128], f16, tag="u")
                    u = ut[:, :F * 128]
                    nc.vector.tensor_add(u[:, :F_LO * 128], lo_ap,
                                         xr_ap[:, :F_LO * 128])
                    nc.vector.tensor_add(u[:, F_LO * 128:F * 128], hi_ap,
                                         xr_ap[:, F_LO * 128:F * 128])
                    # leaky relu: g = 0.6*(u + |(2/3)u|), 0.6 folded in att;
                    # |.| runs on the Activation engine.
                    tt_ = pc.tile([128, FMAX * 128], f16, tag="t")
                    t = tt_[:, :F * 128]
                    nc.scalar.activation(t, u, Act.Abs, scale=2.0 / 3.0)
                    nc.vector.tensor_add(u, u, t)
                    # ga = g * att  (broadcast att row over tiles, packed)
                    nc.vector.tensor_tensor(
                        t.rearrange("p (f x) -> p f x", f=F),
                        u.rearrange("p (f x) -> p f x", f=F),
                        attb_sb[:].unsqueeze(1).broadcast_to([128, F, 128]),
                        op=Alu.mult)
                    # e4 = sum over C: binary tree, all fp16 packed 2x
                    t4 = t.rearrange("p (f h c) -> p f h c", h=H, c=C)
                    r16 = pc.tile([128, FMAX * H * 16], f16, tag="r16")
                    v16 = r16[:, :F * H * 16].rearrange(
                        "p (f h c) -> p f h c", h=H, c=16)
                    nc.vector.tensor_add(v16, t4[:, :, :, 0:16],
                                         t4[:, :, :, 16:32])
                    r8 = pc.tile([128, FMAX * H * 8], f16, tag="r8")
                    v8 = r8[:, :F * H * 8].rearrange(
                        "p (f h c) -> p f h c", h=H, c=8)
                    nc.vector.tensor_add(v8, v16[:, :, :, 0:8],
                                         v16[:, :, :, 8:16])
                    r4 = pc.tile([128, FMAX * H * 4], f16, tag="r4")
                    v4 = r4[:, :F * H * 4].rearrange(
                        "p (f h c) -> p f h c", h=H, c=4)
                    nc.vector.tensor_add(v4, v8[:, :, :, 0:4],
                                         v8[:, :, :, 4:8])
                    r2 = pc.tile([128, FMAX * H * 2], f16, tag="r2")
                    v2 = r2[:, :F * H * 2].rearrange(
                        "p (f h c) -> p f h c", h=H, c=2)
                    nc.vector.tensor_add(v2, v4[:, :, :, 0:2],
                                         v4[:, :, :, 2:4])
                    e4t = pc.tile([128, FMAX * H], f16, tag="e4")
                    e4 = e4t[:, :F * H]
                    ve = e4.rearrange("p (f h) -> p f h", h=H)
                    nc.vector.tensor_tensor(
                        ve.unsqueeze(3), v2[:, :, :, 0:1],
                        v2[:, :, :, 1:2], op=Alu.add)
                    # clamp (fp16-range guard); shift folded into exp bias
                    nc.vector.tensor_scalar(e4, e4, E_CLAMP, None,
                                            op0=Alu.min)
                    # ex expanded over C on the Activation engine
                    exxt = pc.tile([128, FMAX * 128], f16, tag="exx")
                    exx = exxt[:, :F * 128]
                    nc.scalar.activation(
                        exx.rearrange("p (f h c) -> p f h c", h=H, c=C),
                        ve.unsqueeze(3).broadcast_to([128, F, H, C]),
                        Act.Exp, bias=eshift_sb[:, 0:1])
                    msgext = pc.tile([128, FMAX * 132], f16, tag="msgex")
                    mv = msgext[:, :F * 132].rearrange(
                        "p (f x) -> p f x", x=132)
                    # denom columns: plain ex [p, f, h]
                    nc.scalar.activation(
                        mv[:, :, 128:132], ve, Act.Exp,
                        bias=eshift_sb[:, 0:1])
                    oh_ap = oh_tiles[b // MERGE][
                        :, (OF[b] - OF[(b // MERGE) * MERGE]) * 128:
                        (OF[b + 1] - OF[(b // MERGE) * MERGE]) * 128]
                    pending.append((b, lo_ap, hi_ap, exx, mv, oh_ap))

                def _stage2(state):
                    b, lo_ap, hi_ap, exx, mv, oh_ap = state
                    F_LO, F_HI, F = F_lo_b[b], F_hi_b[b], F_b[b]
                    # msg = xl * ex (2x; read xl from the merged tiles)
                    mm = mv[:, :, 0:128]
                    nc.vector.tensor_tensor(
                        mm[:, 0:F_LO, :],
                        lo_ap.rearrange("p (f x) -> p f x", x=128),
                        exx[:, :F_LO * 128]
                        .rearrange("p (f x) -> p f x", x=128), op=Alu.mult)
                    nc.vector.tensor_tensor(
                        mm[:, F_LO:F, :],
                        hi_ap.rearrange("p (f x) -> p f x", x=128),
                        exx[:, F_LO * 128:F * 128]
                        .rearrange("p (f x) -> p f x", x=128), op=Alu.mult)
                    # scatter-add into PSUM via one-hot matmul
                    acc = pc_ps.tile([128, 132], f32, tag="acc")
                    for tt in range(F):
                        nc.tensor.matmul(acc[:],
                                         lhsT=oh_ap[:, tt * 128:(tt + 1) * 128],
                                         rhs=mv[:, tt, :],
                                         start=(tt == 0), stop=(tt == F - 1))
                    nc.scalar.activation(stg[:, b * 132:(b + 1) * 132],
                                         acc[:], Act.Copy)

                xr_tiles = {}
                oh_tiles = {}

                def _emit_xr_oh(g):
                    gb0 = g * MERGE
                    gm = min(MERGE, NB - gb0)
                    gnf = OF[gb0 + gm] - OF[gb0]
                    xrb = gp.tile([128, MERGE * FMAX * 128], f16, tag="xrb")
                    nc.gpsimd.dma_gather(
                        out_ap=xrb[:, :gnf * 128]
                        .rearrange("p (f x) -> p f x", x=128),
                        in_ap=xr_loc.ap(),
                        idxs_ap=gixr_sb[:, OF[gb0] * 8:OF[gb0 + gm] * 8],
                        num_idxs=gnf * 128, num_idxs_reg=gnf * 128,
                        elem_size=HC, single_packet=False)
                    xr_tiles[g] = xrb
                    ohs = gp.tile([128, MERGE * FMAX * 128], dt.float8e4,
                                  tag="ohs")
                    nc.sync.dma_start(
                        ohs[:, :gnf * 128],
                        ohp.ap()[:, OF[gb0] * 128:OF[gb0 + gm] * 128])
                    oh_tiles[g] = ohs

                for mg in range(NMG if "c" in phases else 0):
                    b0 = mg * MERGE
                    m = min(MERGE, NB - b0)
                    nlo = OLO[b0 + m] - OLO[b0]
                    nhi = OHI[b0 + m] - OHI[b0]
                    # xr + one-hot staged one group ahead (and the first two
                    # groups land inside the AllGather window)
                    if mg == 0:
                        _emit_xr_oh(0)
                    if mg + 1 < NMG:
                        _emit_xr_oh(mg + 1)
                    xlo = gp.tile([128, MERGE * FLOMAX * 128], f16, tag="xlo")
                    nc.gpsimd.dma_gather(
                        out_ap=xlo[:, :nlo * 128]
                        .rearrange("p (f x) -> p f x", x=128),
                        in_ap=xl_full.ap(),
                        idxs_ap=gilo_sb[:, OLO[b0] * 8:OLO[b0 + m] * 8],
                        num_idxs=nlo * 128, num_idxs_reg=nlo * 128,
                        elem_size=HC, single_packet=False)
                    xhi = gp.tile([128, MERGE * FHIMAX * 128], f16, tag="xhi")
                    nc.gpsimd.dma_gather(
                        out_ap=xhi[:, :nhi * 128]
                        .rearrange("p (f x) -> p f x", x=128),
                        in_ap=xl_full.ap()[SPLIT:NTOT, :],
                        idxs_ap=gihi_sb[:, OHI[b0] * 8:OHI[b0 + m] * 8],
                        num_idxs=nhi * 128, num_idxs_reg=nhi * 128,
                        elem_size=HC, single_packet=False)
                    xrb = xr_tiles.pop(mg)

                    for bi in range(m):
                        _stage1(b0 + bi, b0, xlo, xhi, xrb)
                        if len(pending) == 2:
                            _stage2(pending.pop(0))
                if "c" in phases:
                    while pending:
                        _stage2(pending.pop(0))

            pc_ps_ctx.__exit__(None, None, None)
            pc_ctx.__exit__(None, None, None)
            gp_ctx.__exit__(None, None, None)
            with tc.tile_pool(name="ep", bufs=2) as ep:
                # ---- batched epilogue, two halves (tail overlap) ----
                if "e" in phases:
                    for h0, h1 in ((0, NB // 2), (NB // 2, NB)):
                        nb = h1 - h0
                        sv = stg[:, h0 * 132:h1 * 132].rearrange(
                            "p (b x) -> p b x", b=nb)
                        dn = ep.tile([128, NB // 2 * 4], f32, tag="dn")
                        nc.vector.tensor_scalar(
                            dn[:].rearrange("p (b h) -> p b h", b=nb),
                            sv[:, :, 128:132], 1e-6, None, op0=Alu.add)
                        rc = ep.tile([128, NB // 2 * 4], f32, tag="rc")
                        nc.vector.reciprocal(rc[:], dn[:])
                        # keep 1/denom inside fp16 range for the expansion
                        nc.vector.tensor_scalar(rc[:], rc[:], 60000.0, None,
                                                op0=Alu.min)
                        # expand 1/denom over C on the Activation engine
                        rcx = ep.tile([128, NB // 2 * 128], f16, tag="rcx")
                        nc.scalar.activation(
                            rcx[:].rearrange("p (b h c) -> p b h c", b=nb, c=C),
                            rc[:].rearrange("p (b h) -> p b h", b=nb)
                            .unsqueeze(3).broadcast_to([128, nb, H, C]),
                            Act.Copy)
                        o1 = ep.tile([128, NB // 2 * 128], f16, tag="o1")
                        o13 = o1[:].rearrange("p (b x) -> p b x", b=nb)
                        nc.vector.tensor_tensor(
                            o13, sv[:, :, 0:128],
                            rcx[:].rearrange("p (b x) -> p b x", b=nb),
                            op=Alu.mult)
                        nc.vector.tensor_tensor(
                            o13, o13,
                            bgat_sb[:].unsqueeze(1).broadcast_to([128, nb, 128]),
                            op=Alu.add)
                        # elu(x) = (max(x,0) - 1) + exp(min(x,0))
                        o2 = ep.tile([128, NB // 2 * 128], f16, tag="o2")
                        nc.vector.tensor_scalar(o2[:], o1[:], 0.0, None,
                                                op0=Alu.min)
                        nc.scalar.activation(o2[:], o2[:], Act.Exp)
                        nc.vector.tensor_scalar(o1[:], o1[:], 0.0, -1.0,
                                                op0=Alu.max, op1=Alu.add)
                        nc.vector.tensor_add(o1[:], o1[:], o2[:])
                        nc.sync.dma_start(
                            outp.ap()[h0 * 128:h1 * 128, :]
                            .rearrange("(b p) j -> p b j", p=128),
                            o1[:].rearrange("p (b j) -> p b j", b=nb))
            stgp_ctx.__exit__(None, None, None)

    nc.compile()
    return nc


# ---------------------------------------------------------------------------
# entry point
# ---------------------------------------------------------------------------

def _make_in_maps(inputs):
    x_mrna = np.asarray(inputs["x_mrna"], np.float32)
    x_mirna = np.asarray(inputs["x_mirna"], np.float32)
    att = np.asarray(inputs["att"], np.float32)
    edge_index = np.asarray(inputs["edge_index"])

    edge_arrays, F_lo_b, F_hi_b = prep_edges(edge_index)
    shards = prep_shards(x_mrna, x_mirna)
    FMAX = max(a + b for a, b in zip(F_lo_b, F_hi_b))

    # leaky(u) = 0.6*u + 0.4*|u| = 0.6*(u + |(2/3)*u|); fold 0.6 into att
    att_flat = att.reshape(HC) * 0.6
    attb = np.tile(att_flat[None, :], (128, 1)).astype(np.float16)
    bgatb = np.tile(np.asarray(inputs["b_gat"], np.float32)[None, :],
                    (128, 1)).astype(np.float16)

    def padw(w, dpad):
        w = np.asarray(w, np.float32)
        out = np.zeros((dpad, w.shape[1]), np.float16)
        out[:w.shape[0]] = w.astype(np.float16)
        return out

    common = dict(
        wp1=padw(inputs["Wp1"], D1PAD),
        bp1=np.asarray(inputs["bp1"], np.float32).reshape(P, 1),
        wp2=padw(inputs["Wp2"], D2PAD),
        bp2=np.asarray(inputs["bp2"], np.float32).reshape(P, 1),
        wl=np.asarray(inputs["Wl"], np.float32).astype(np.float16),
        wr=np.asarray(inputs["Wr"], np.float32).astype(np.float16),
        attb=attb, bgat=bgatb)

    in_maps = []
    for c in range(CORES):
        xmc, xrc = shards[c]
        m = dict(common)
        m.update(xm=xmc, xmi=xrc, **edge_arrays[c])
        in_maps.append(m)
    return in_maps, F_lo_b, F_hi_b


def _assemble(results):
    out = np.empty((N1 + N2, HC), np.float32)
    for c in range(CORES):
        o = np.asarray(results[c]["outp"], np.float32)
        out[c * N1PC:(c + 1) * N1PC] = o[:N1PC]
        out[N1 + c * N2PC:N1 + (c + 1) * N2PC] = o[N1PAD:N1PAD + N2PC]
    return out


def kernel(**inputs):
    from concourse.bass_utils import run_bass_kernel_spmd

    in_maps, F_LO, F_HI = _make_in_maps(inputs)
    nc = build_program(F_LO, F_HI)
    res = run_bass_kernel_spmd(nc, in_maps, list(range(CORES)))
    return _assemble(res.results)


if __name__ == "__main__":
    rng = np.random.default_rng(0)
    E = 800000
    ei = rng.integers(0, N1 + N2, size=(2, E), dtype=np.int32)
    arrs, flo, fhi = prep_edges(ei)
    print("sum F_lo", sum(flo), "sum F_hi", sum(fhi))
